# revision 8
# baseline (speedup 1.0000x reference)
"""Multi-head attention Trainium2 kernel (Bass/Tile), 8-core data-parallel.

Problem: B=8, N=2048, E=768, H=8 heads, D=96.
  q = x@Wq+bq; k = x@Wk+bk; v = x@Wv+bv  (per batch)
  energy = q @ k^T per head; att = softmax(energy)/sqrt(E); out = (att@v)@Wo + bo

Sharding: data-parallel over batch — each of the 8 cores handles one batch
element with a full copy of the weights. No collectives.

Per-core algorithm (all matmuls bf16 with fp32 PSUM accumulation):
  - PE warmup: ~10 dummy matmuls on a zeroed tile at kernel start so the HAM
    clock-gate reaches K=8/8 (~3.4us) while the startup DMAs land; without it
    the trickle-fed projection matmuls run at 1.2 GHz for ~28us.
  - x^T [E, N] is DMA'd in (host pre-transposes + casts bf16).
  - Q^T_h = Wq_h^T @ x^T + bq_h  per head  [96, 2048]   (bias = per-partition DVE add)
  - K^T_h = Wk_h^T @ x^T                   [96, 2048]   (bk dropped: softmax shift-invariant)
  - V' [N, 8*97]: per head block = [ones column | 96 data cols (x@Wv)].
  - Per head, per 1024-wide q window pair:
      energy^T[k_chunk, q] = (K^T_h chunk)^T @ Q^T_h    -> PSUM [128, 2x512]
      att = exp(energy^T)  (one ACT instr per [128,1024]; no max subtraction
        needed: |energy| < ~20 so fp32/bf16 exp cannot overflow)
      out'^T [97, 512] += V'_h[k_chunk]^T @ att          (row 0 = softmax denominator)
      rb = 1/out'^T  (reciprocal_approx_fast; only row 0 -- the denominator --
        is consumed)
      rbb[0:97] = partition_broadcast(rb[0])  (GpSimd, HW broadcasts partition 0)
      normalized rows are written STRAIGHT INTO the merged onorm layout:
        onorm6[t] [128, 2048] bf16, t=0..5, rows = E-dim (heads packed at 96h),
        via 1-2 partition-shifted DVE muls per window (head rows straddle the
        128-row tile boundary for h not divisible by 4).
  - Next head's Q/K projection matmuls are interleaved into the attention loop
    so the in-order PE fills its slack while ACT (exp) is the local bottleneck;
    the last head's slack is filled with the first half of the output projection.
  - Final: out[n_chunk, :] = sum_t onorm6[t][:, n_chunk]^T @ (Wo/sqrt(E))[t],
    full K=128 contraction (6 matmuls per 512-wide half instead of 8 per-head
    K=97 matmuls). Output copied to bf16 SBUF, stored bf16 (host casts to f32).
  - Host adds bo_eff = bo + bv @ Wo / sqrt(E)  (exact because softmax rows sum to 1).
"""

import math
import sys
import types

import numpy as np
import ml_dtypes

B, N, E, H = 8, 2048, 768, 8
D = E // H          # 96
DP = D + 1          # 97: per-head V width incl. leading ones column
N_CORES = 8
NT = N // 128       # 16 row chunks of x / V
ET = E // 128       # 6 embedding chunks
QF = 512            # moving free-dim tile
NQF = N // QF       # 4 q windows
NQP = NQF // 2      # 2 q window pairs

_BF16 = ml_dtypes.bfloat16

_compiled = {}

# head h's 96 output rows land at E-rows [96h, 96h+96): list of
# (onorm_tile, row_start, row_end, offset_into_head) pieces. Partition
# accesses must obey the HW quadrant mux: spans <=32 rows may start at any
# multiple of 32, spans <=64 only at 0/64, wider spans only at 0 — for BOTH
# the destination row range and the po source row range. (The ones column
# sits at position 96 of each V' block so po data rows are 0-95.)


def _legal(start, cnt):
    if cnt <= 32:
        return start % 32 == 0
    if cnt <= 64:
        return start in (0, 64)
    return start == 0


_HEAD_PIECES = []
for _h in range(H):
    _pieces = []
    _d = 0
    while _d < D:
        _row = D * _h + _d
        _t = _row // 128
        _r0 = _row % 128
        _cnt = min(D - _d, 128 - _r0)
        while _cnt > 32 and not (_legal(_r0, _cnt) and _legal(_d, _cnt)):
            _cnt -= 32
        _pieces.append((_t, _r0, _r0 + _cnt, _d))
        _d += _cnt
    _HEAD_PIECES.append(_pieces)


def _install_ntff_hook_stub():
    """bass_utils imports antenv.axon_hooks when tracing; provide the glue if
    the image's antenv stub lacks it (harmless when trace=False)."""
    if "antenv.axon_hooks" in sys.modules:
        return
    hook = None
    try:
        from trn_agent_boot.trn_boot import _ntff_profile_via_ctypes

        hook = _ntff_profile_via_ctypes("/opt/axon/libaxon_pjrt.so")
    except Exception:
        pass
    mod = types.ModuleType("antenv.axon_hooks")
    mod.get_axon_ntff_profile_hook = lambda: hook
    mod.set_axon_ntff_profile_hook = lambda h: None
    sys.modules["antenv.axon_hooks"] = mod


def _build():
    import concourse.tile as tile
    import concourse.bacc as bacc
    from concourse import mybir

    bf = mybir.dt.bfloat16
    f32 = mybir.dt.float32
    Exp = mybir.ActivationFunctionType.Exp

    nc = bacc.Bacc("TRN2", target_bir_lowering=False, debug=False,
                   num_devices=N_CORES)

    xT_d = nc.dram_tensor("xT", [E, N], bf, kind="ExternalInput")
    wq_d = nc.dram_tensor("wq", [E, E], bf, kind="ExternalInput")
    wk_d = nc.dram_tensor("wk", [E, E], bf, kind="ExternalInput")
    wv_d = nc.dram_tensor("wv", [E, E], bf, kind="ExternalInput")
    wo_d = nc.dram_tensor("wo", [E, E], bf, kind="ExternalInput")  # pre-scaled 1/sqrt(E)
    bq_d = nc.dram_tensor("bq", [D, H], f32, kind="ExternalInput")
    out_d = nc.dram_tensor("out", [N, E], bf, kind="ExternalOutput")

    with tile.TileContext(nc) as tc:
        from contextlib import ExitStack

        with ExitStack() as ctx:
            const = ctx.enter_context(tc.tile_pool(name="const", bufs=1))
            vpool = ctx.enter_context(tc.tile_pool(name="vstore", bufs=1))
            qkpool = ctx.enter_context(tc.tile_pool(name="qk", bufs=2))
            onpool = ctx.enter_context(tc.tile_pool(name="onorm", bufs=1))
            att_pool = ctx.enter_context(tc.tile_pool(name="att", bufs=3))
            small = ctx.enter_context(tc.tile_pool(name="small", bufs=4))
            outsb_pool = ctx.enter_context(tc.tile_pool(name="outsb", bufs=3))

            # ---- PE warmup: spin dummy matmuls so the HAM un-throttles the
            # PE clock (1.2 -> 2.4 GHz needs ~3.4us of sustained activity)
            # while the startup DMAs land.
            warm = const.tile([128, QF], bf, tag="warm", name="warm")
            nc.vector.memset(warm[:], 0.0)
            with tc.tile_pool(name="warmpsum", bufs=1, space="PSUM") as wpsum:
                pw = wpsum.tile([128, QF], f32, tag="pw", name="pw")
                for r in range(10):
                    nc.tensor.matmul(pw[:], warm[:, 0:128], warm[:],
                                     start=True, stop=True)

            # ---- persistent SBUF loads ----
            # Critical-path loads (wq / xT window 0) go on the two HWDGE
            # queues (sync, scalar) only; later tiles fan out to gpsimd's
            # software DGE as well. Each dma_start costs ~590ns of issue time
            # on its queue engine, so small loads are batched (bq is one DMA).
            xTw = [[const.tile([128, QF], bf, tag=f"xT{i}_{w}", name=f"xT{i}_{w}")
                    for w in range(NQF)] for i in range(ET)]

            wq = [const.tile([128, E], bf, tag=f"wq{i}", name=f"wq{i}")
                  for i in range(ET)]
            for i in range(ET):
                nc.sync.dma_start(wq[i][:], wq_d.ap()[i * 128:(i + 1) * 128, :])
                nc.scalar.dma_start(xTw[i][0][:],
                                    xT_d.ap()[i * 128:(i + 1) * 128, 0:QF])
            bq_sb = const.tile([D, H], f32, tag="bq", name="bq")
            nc.gpsimd.dma_start(bq_sb[:], bq_d.ap()[:, :])

            def load_w(dram, name, qs):
                tiles = []
                for i in range(ET):
                    t = const.tile([128, E], bf, tag=f"{name}{i}", name=f"{name}{i}")
                    qs[i % len(qs)].dma_start(
                        t[:], dram.ap()[i * 128:(i + 1) * 128, :])
                    tiles.append(t)
                return tiles

            wk = load_w(wk_d, "wk", [nc.sync, nc.scalar])
            wv = load_w(wv_d, "wv", [nc.sync, nc.scalar])
            for w in range(1, NQF):
                for i in range(ET):
                    q = [nc.sync, nc.scalar, nc.gpsimd][(i + w) % 3]
                    q.dma_start(xTw[i][w][:],
                                xT_d.ap()[i * 128:(i + 1) * 128,
                                          w * QF:(w + 1) * QF])
            wo = load_w(wo_d, "wo", [nc.gpsimd, nc.sync, nc.scalar])

            # ---- Phases 1+2 ----
            # onorm6: merged normalized-attention layout, rows = E dim
            onorm6 = [onpool.tile([128, N], bf, tag=f"on{t}", name=f"on{t}")
                      for t in range(ET)]
            vtiles = []
            qkpsum_cm = tc.tile_pool(name="qkpsum", bufs=2, space="PSUM")
            with qkpsum_cm as qkpsum:

                def proj_tasks(h, qt, kt):
                    """Micro-tasks for head h's Q^T/K^T projections: one matmul
                    (or finishing DVE op) per yield. Window-interleaved to match
                    the startup DMA arrival order (wq/xT0 first, then wk, then
                    later xT windows)."""
                    for qf in range(NQF):
                        for dst, w, bias in ((qt, wq, bq_sb), (kt, wk, None)):
                            pq = qkpsum.tile([D, QF], f32, tag="pqk",
                                             name=f"pqk{h}_{qf}_{0 if bias is not None else 1}")
                            for ein in range(ET):
                                nc.tensor.matmul(
                                    pq[:],
                                    w[ein][:, h * D:(h + 1) * D],
                                    xTw[ein][qf][:],
                                    start=(ein == 0), stop=(ein == ET - 1),
                                )
                                yield
                            sl = dst[:, qf * QF:(qf + 1) * QF]
                            if bias is not None:
                                nc.vector.tensor_scalar_add(sl, pq[:],
                                                            bias[:, h:h + 1])
                            else:
                                nc.vector.tensor_copy(sl, pq[:])
                            yield

                def attention(h, qt, kt, next_tasks, epsum, opsum,
                              defer_fill_first_pair=False):
                    """Head h attention; drains next_tasks (next head's
                    projections, or the tail of the output projection) between
                    inner iterations to fill PE slack."""
                    def drain(k, qp=1):
                        if defer_fill_first_pair and qp == 0:
                            return
                        for _ in range(k):
                            if next_tasks is None:
                                return
                            if next(next_tasks, "done") == "done":
                                return

                    for qp in range(NQP):
                        po = [opsum.tile([DP, QF], f32, tag="po",
                                         name=f"po{h}_{qp}_{j}")
                              for j in range(2)]
                        for kc in range(NT):
                            pe = epsum.tile([128, 2 * QF], f32, tag="pe",
                                            name=f"pe{h}_{qp}_{kc}")
                            for j in range(2):
                                nc.tensor.matmul(
                                    pe[:, j * QF:(j + 1) * QF],
                                    kt[:, kc * 128:(kc + 1) * 128],
                                    qt[:, (2 * qp + j) * QF:(2 * qp + j + 1) * QF],
                                    start=True, stop=True,
                                )
                            att = att_pool.tile([128, 2 * QF], bf, tag="att",
                                                name=f"att{h}_{qp}_{kc}")
                            nc.scalar.activation(att[:], pe[:], Exp)
                            for j in range(2):
                                nc.tensor.matmul(
                                    po[j][:],
                                    vtiles[kc][:, h * DP:(h + 1) * DP],
                                    att[:, j * QF:(j + 1) * QF],
                                    start=(kc == 0), stop=(kc == NT - 1),
                                )
                            drain(2, qp)
                        for j in range(2):
                            qf = 2 * qp + j
                            # denominator row (po row 96) to partition 0 via a
                            # plain copy (custom-DVE ops and partition_broadcast
                            # do not honor shifted input bases on HW), then
                            # reciprocal + broadcast from partition 0.
                            den = small.tile([1, QF], f32, tag="den",
                                             name=f"den{h}_{qf}")
                            nc.vector.tensor_copy(den[0:1, :], po[j][D:DP, :])
                            rb = small.tile([1, QF], f32, tag="rb",
                                            name=f"rb{h}_{qf}")
                            nc.vector.reciprocal_approx_fast(rb[0:1, :],
                                                             den[0:1, :])
                            rbb = small.tile([128, QF], f32, tag="rbb",
                                             name=f"rbb{h}_{qf}")
                            nc.gpsimd.partition_broadcast(rbb[:], rb[0:1, :])
                            for (t, r0, r1, off) in _HEAD_PIECES[h]:
                                cnt = r1 - r0
                                nc.vector.tensor_mul(
                                    onorm6[t][r0:r1, qf * QF:(qf + 1) * QF],
                                    po[j][off:off + cnt, :],
                                    rbb[r0:r1, :])
                            drain(1, qp)

                # head 0 projections + V' phase, interleaved window-by-window
                # so the in-order PE consumes tiles in DMA arrival order
                # (wq/xT0, wk, wv, xT1, xT2, xT3).
                qts, kts = {}, {}
                qts[0] = qkpool.tile([D, N], bf, tag="qt", name="qt0")
                kts[0] = qkpool.tile([D, N], bf, tag="kt", name="kt0")
                p0 = proj_tasks(0, qts[0], kts[0])

                def drain_p0(k):
                    for _ in range(k):
                        if next(p0, "done") == "done":
                            return

                with tc.tile_pool(name="vpsum", bufs=2, space="PSUM") as vpsum:
                    def emit_v_chunk(nch):
                        pv = vpsum.tile([128, E], f32, tag="pv", name=f"pv{nch}")
                        for f0, f1 in ((0, 512), (512, 768)):
                            for ein in range(ET):
                                nc.tensor.matmul(
                                    pv[:, f0:f1],
                                    xTw[ein][nch // 4][:, (nch % 4) * 128:
                                                       (nch % 4 + 1) * 128],
                                    wv[ein][:, f0:f1],
                                    start=(ein == 0), stop=(ein == ET - 1),
                                )
                        vt = vpool.tile([128, H * DP], bf, tag=f"v{nch}",
                                        name=f"v{nch}")
                        vview = vt[:].rearrange("p (h c) -> p h c", c=DP)
                        nc.vector.memset(vview[:, :, D:DP], 1.0)
                        nc.vector.tensor_copy(
                            vview[:, :, 0:D],
                            pv[:].rearrange("p (h c) -> p h c", c=D),
                        )
                        vtiles.append(vt)

                    for w in range(NQF):
                        drain_p0(7)           # half a window group of q+k tasks
                        emit_v_chunk(4 * w)
                        emit_v_chunk(4 * w + 1)
                        drain_p0(7)
                        emit_v_chunk(4 * w + 2)
                        emit_v_chunk(4 * w + 3)
                    for _ in p0:
                        pass

                store_q = [nc.sync, nc.scalar, nc.gpsimd]

                def final_tasks(nchs):
                    """Output-projection micro-tasks: one matmul (or the
                    finishing copy/store) per yield. Full K=128 contraction
                    over the merged onorm tiles (6 matmuls per 512-wide half).
                    PSUM comes from the qkpsum pool's slots (idle once
                    projections are done)."""
                    for nch in nchs:
                        osb = outsb_pool.tile([128, E], bf, tag="osb",
                                              name=f"osb{nch}")
                        for f0, f1 in ((0, 512), (512, 768)):
                            pf = qkpsum.tile([128, f1 - f0], f32, tag="pqk",
                                             name=f"pf{nch}_{f0}")
                            for t in range(ET):
                                nc.tensor.matmul(
                                    pf[:],
                                    onorm6[t][:, nch * 128:(nch + 1) * 128],
                                    wo[t][:, f0:f1],
                                    start=(t == 0), stop=(t == ET - 1),
                                )
                                yield
                            nc.vector.tensor_copy(osb[:, f0:f1], pf[:])
                            yield
                        store_q[nch % 3].dma_start(
                            out_d.ap()[nch * 128:(nch + 1) * 128, :], osb[:])

                final_rest = None
                with tc.tile_pool(name="epsum", bufs=2, space="PSUM") as epsum, \
                     tc.tile_pool(name="opsum", bufs=2, space="PSUM") as opsum:
                    for h in range(H):
                        if h + 1 < H:
                            qts[h + 1] = qkpool.tile([D, N], bf, tag="qt",
                                                     name=f"qt{h+1}")
                            kts[h + 1] = qkpool.tile([D, N], bf, tag="kt",
                                                     name=f"kt{h+1}")
                            tasks = proj_tasks(h + 1, qts[h + 1], kts[h + 1])
                        else:
                            # last head: fill PE slack with the first half of
                            # the output projection (n-chunks 0..7 only need
                            # head-7 windows 0/1, normalized in window pair 0).
                            tasks = final_tasks(range(8))
                        attention(h, qts[h], kts[h], tasks, epsum, opsum,
                                  defer_fill_first_pair=(h + 1 == H))
                        if tasks is not None:
                            for _ in tasks:  # finish any leftovers
                                pass
                        qts.pop(h), kts.pop(h)
                    final_rest = final_tasks(range(8, NT))
                    for _ in final_rest:
                        pass


    nc.compile()
    return nc


def _get_nc():
    if "nc" not in _compiled:
        _install_ntff_hook_stub()
        _compiled["nc"] = _build()
    return _compiled["nc"]


def prepare_in_maps(x, Wq, Wk, Wv, Wo, bq):
    """Host-side prep: transpose/cast per-core inputs."""
    scale = np.float32(1.0 / math.sqrt(E))
    wq_b = np.ascontiguousarray(Wq.astype(_BF16))
    wk_b = np.ascontiguousarray(Wk.astype(_BF16))
    wv_b = np.ascontiguousarray(Wv.astype(_BF16))
    wo_b = np.ascontiguousarray((Wo.astype(np.float32) * scale).astype(_BF16))
    bq_c = np.ascontiguousarray(
        bq.astype(np.float32).reshape(H, D).T)  # [D, H], col h = head h bias
    in_maps = []
    for c in range(N_CORES):
        in_maps.append({
            "xT": np.ascontiguousarray(x[c].T.astype(_BF16)),
            "wq": wq_b, "wk": wk_b, "wv": wv_b, "wo": wo_b,
            "bq": bq_c,
        })
    return in_maps


def run(x, Wq, bq, Wk, bk, Wv, bv, Wo, bo, trace=False, **spmd_kwargs):
    """Run on hardware; returns (out [B,N,E] fp32, BassKernelResults)."""
    from concourse.bass_utils import run_bass_kernel_spmd

    nc = _get_nc()
    in_maps = prepare_in_maps(x, Wq, Wk, Wv, Wo, bq)
    res = run_bass_kernel_spmd(nc, in_maps, core_ids=list(range(N_CORES)),
                               trace=trace, **spmd_kwargs)
    scale = np.float32(1.0 / math.sqrt(E))
    bo_eff = (bo.astype(np.float32)
              + (bv.astype(np.float32) @ Wo.astype(np.float32)) * scale)
    out = np.stack([res.results[c]["out"].astype(np.float32)
                    for c in range(N_CORES)], axis=0)
    out = out + bo_eff[None, None, :]
    return out.astype(np.float32), res


def kernel(x, Wq, bq, Wk, bk, Wv, bv, Wo, bo):
    x = np.asarray(x); Wq = np.asarray(Wq); bq = np.asarray(bq)
    Wk = np.asarray(Wk); bk = np.asarray(bk); Wv = np.asarray(Wv)
    bv = np.asarray(bv); Wo = np.asarray(Wo); bo = np.asarray(bo)
    out, _ = run(x, Wq, bq, Wk, bk, Wv, bv, Wo, bo, trace=False)
    return out


# revision 12
# speedup vs baseline: 1.0367x; 1.0367x over previous
"""Multi-head attention Trainium2 kernel (Bass/Tile), 8-core data-parallel.

Problem: B=8, N=2048, E=768, H=8 heads, D=96.
  q = x@Wq+bq; k = x@Wk+bk; v = x@Wv+bv  (per batch)
  energy = q @ k^T per head; att = softmax(energy)/sqrt(E); out = (att@v)@Wo + bo

Sharding: data-parallel over batch — each of the 8 cores handles one batch
element with a full copy of the weights. No collectives.

Per-core algorithm (all matmuls bf16 with fp32 PSUM accumulation):
  - PE warmup: ~10 dummy matmuls on a zeroed tile at kernel start so the HAM
    clock-gate reaches K=8/8 (~3.4us) while the startup DMAs land; without it
    the trickle-fed projection matmuls run at 1.2 GHz for ~28us.
  - x^T [E, N] is DMA'd in (host pre-transposes + casts bf16).
  - Q^T_h = Wq_h^T @ x^T + bq_h  per head  [96, 2048]   (bias = per-partition DVE add)
  - K^T_h = Wk_h^T @ x^T                   [96, 2048]   (bk dropped: softmax shift-invariant)
  - V' [N, 8*97]: per head block = [ones column | 96 data cols (x@Wv)].
  - Per head, per 1024-wide q window pair:
      energy^T[k_chunk, q] = (K^T_h chunk)^T @ Q^T_h    -> PSUM [128, 2x512]
      att = exp(energy^T)  (one ACT instr per [128,1024]; no max subtraction
        needed: |energy| < ~20 so fp32/bf16 exp cannot overflow)
      out'^T [97, 512] += V'_h[k_chunk]^T @ att          (row 0 = softmax denominator)
      rb = 1/out'^T  (reciprocal_approx_fast; only row 0 -- the denominator --
        is consumed)
      rbb[0:97] = partition_broadcast(rb[0])  (GpSimd, HW broadcasts partition 0)
      normalized rows are written STRAIGHT INTO the merged onorm layout:
        onorm6[t] [128, 2048] bf16, t=0..5, rows = E-dim (heads packed at 96h),
        via 1-2 partition-shifted DVE muls per window (head rows straddle the
        128-row tile boundary for h not divisible by 4).
  - Next head's Q/K projection matmuls are interleaved into the attention loop
    so the in-order PE fills its slack while ACT (exp) is the local bottleneck;
    the last head's slack is filled with the first half of the output projection.
  - Final: out[n_chunk, :] = sum_t onorm6[t][:, n_chunk]^T @ (Wo/sqrt(E))[t],
    full K=128 contraction (6 matmuls per 512-wide half instead of 8 per-head
    K=97 matmuls). Output copied to bf16 SBUF, stored bf16 (host casts to f32).
  - Host adds bo_eff = bo + bv @ Wo / sqrt(E)  (exact because softmax rows sum to 1).
"""

import math
import sys
import types

import numpy as np
import ml_dtypes

B, N, E, H = 8, 2048, 768, 8
D = E // H          # 96
DP = D + 1          # 97: per-head V width incl. leading ones column
N_CORES = 8
NT = N // 128       # 16 row chunks of x / V
ET = E // 128       # 6 embedding chunks
QF = 512            # moving free-dim tile
NQF = N // QF       # 4 q windows
NQP = NQF // 2      # 2 q window pairs

_BF16 = ml_dtypes.bfloat16

_compiled = {}

# head h's 96 output rows land at E-rows [96h, 96h+96): list of
# (onorm_tile, row_start, row_end, offset_into_head) pieces. Partition
# accesses must obey the HW quadrant mux: spans <=32 rows may start at any
# multiple of 32, spans <=64 only at 0/64, wider spans only at 0 — for BOTH
# the destination row range and the po source row range. (The ones column
# sits at position 96 of each V' block so po data rows are 0-95.)


def _legal(start, cnt):
    if cnt <= 32:
        return start % 32 == 0
    if cnt <= 64:
        return start in (0, 64)
    return start == 0


_HEAD_PIECES = []
for _h in range(H):
    _pieces = []
    _d = 0
    while _d < D:
        _row = D * _h + _d
        _t = _row // 128
        _r0 = _row % 128
        _cnt = min(D - _d, 128 - _r0)
        while _cnt > 32 and not (_legal(_r0, _cnt) and _legal(_d, _cnt)):
            _cnt -= 32
        _pieces.append((_t, _r0, _r0 + _cnt, _d))
        _d += _cnt
    _HEAD_PIECES.append(_pieces)


def _install_ntff_hook_stub():
    """bass_utils imports antenv.axon_hooks when tracing; provide the glue if
    the image's antenv stub lacks it (harmless when trace=False)."""
    if "antenv.axon_hooks" in sys.modules:
        return
    hook = None
    try:
        from trn_agent_boot.trn_boot import _ntff_profile_via_ctypes

        hook = _ntff_profile_via_ctypes("/opt/axon/libaxon_pjrt.so")
    except Exception:
        pass
    mod = types.ModuleType("antenv.axon_hooks")
    mod.get_axon_ntff_profile_hook = lambda: hook
    mod.set_axon_ntff_profile_hook = lambda h: None
    sys.modules["antenv.axon_hooks"] = mod


def _build():
    import concourse.tile as tile
    import concourse.bacc as bacc
    from concourse import mybir

    bf = mybir.dt.bfloat16
    f32 = mybir.dt.float32
    Exp = mybir.ActivationFunctionType.Exp

    nc = bacc.Bacc("TRN2", target_bir_lowering=False, debug=False,
                   num_devices=N_CORES)

    # xT stored window-tiled: tile (i, w) = x^T[128i:128(i+1), 512w:512(w+1)]
    # is contiguous in DRAM so DMA packets are 4KB (vs 1KB strided rows).
    xT_d = nc.dram_tensor("xT", [ET * NQF * 128, QF], bf, kind="ExternalInput")
    wq_d = nc.dram_tensor("wq", [E, E], bf, kind="ExternalInput")
    wk_d = nc.dram_tensor("wk", [E, E], bf, kind="ExternalInput")
    wv_d = nc.dram_tensor("wv", [E, E], bf, kind="ExternalInput")
    wo_d = nc.dram_tensor("wo", [E, E], bf, kind="ExternalInput")  # pre-scaled 1/sqrt(E)
    bq_d = nc.dram_tensor("bq", [D, H], f32, kind="ExternalInput")
    out_d = nc.dram_tensor("out", [N, E], bf, kind="ExternalOutput")

    with tile.TileContext(nc) as tc:
        from contextlib import ExitStack

        with ExitStack() as ctx:
            const = ctx.enter_context(tc.tile_pool(name="const", bufs=1))
            vpool = ctx.enter_context(tc.tile_pool(name="vstore", bufs=1))
            qkpool = ctx.enter_context(tc.tile_pool(name="qk", bufs=2))
            onpool = ctx.enter_context(tc.tile_pool(name="onorm", bufs=1))
            att_pool = ctx.enter_context(tc.tile_pool(name="att", bufs=3))
            small = ctx.enter_context(tc.tile_pool(name="small", bufs=4))
            outsb_pool = ctx.enter_context(tc.tile_pool(name="outsb", bufs=3))

            # ---- PE warmup: spin dummy matmuls so the HAM un-throttles the
            # PE clock (1.2 -> 2.4 GHz needs ~3.4us of sustained activity)
            # while the startup DMAs land.
            warm = const.tile([128, QF], bf, tag="warm", name="warm")
            nc.vector.memset(warm[:], 0.0)
            with tc.tile_pool(name="warmpsum", bufs=1, space="PSUM") as wpsum:
                pw = wpsum.tile([128, QF], f32, tag="pw", name="pw")
                for r in range(10):
                    nc.tensor.matmul(pw[:], warm[:, 0:128], warm[:],
                                     start=True, stop=True)

            # ---- persistent SBUF loads ----
            # Critical-path loads (wq / xT window 0) go on the two HWDGE
            # queues (sync, scalar) only; later tiles fan out to gpsimd's
            # software DGE as well. Each dma_start costs ~590ns of issue time
            # on its queue engine, so small loads are batched (bq is one DMA).
            xTw = [[const.tile([128, QF], bf, tag=f"xT{i}_{w}", name=f"xT{i}_{w}")
                    for w in range(NQF)] for i in range(ET)]

            def ld_xt(i, w, q):
                r = (i * NQF + w) * 128
                q.dma_start(xTw[i][w][:], xT_d.ap()[r:r + 128, :])

            wq = [const.tile([128, E], bf, tag=f"wq{i}", name=f"wq{i}")
                  for i in range(ET)]
            for i in range(ET):
                nc.sync.dma_start(wq[i][:], wq_d.ap()[i * 128:(i + 1) * 128, :])
                ld_xt(i, 0, nc.scalar)
            bq_sb = const.tile([D, H], f32, tag="bq", name="bq")
            nc.gpsimd.dma_start(bq_sb[:], bq_d.ap()[:, :])

            def load_w(dram, name, qs):
                tiles = []
                for i in range(ET):
                    t = const.tile([128, E], bf, tag=f"{name}{i}", name=f"{name}{i}")
                    qs[i % len(qs)].dma_start(
                        t[:], dram.ap()[i * 128:(i + 1) * 128, :])
                    tiles.append(t)
                return tiles

            wk = load_w(wk_d, "wk", [nc.sync, nc.gpsimd])
            for i in range(ET):
                ld_xt(i, 1, nc.scalar)
            wv = load_w(wv_d, "wv", [nc.sync, nc.gpsimd])
            for w in range(2, NQF):
                for i in range(ET):
                    ld_xt(i, w, [nc.scalar, nc.sync, nc.gpsimd][(i + w) % 3])
            wo = load_w(wo_d, "wo", [nc.gpsimd, nc.sync, nc.scalar])

            # ---- Phases 1+2 ----
            # onorm6: merged normalized-attention layout, rows = E dim
            onorm6 = [onpool.tile([128, N], bf, tag=f"on{t}", name=f"on{t}")
                      for t in range(ET)]
            vtiles = []
            qkpsum_cm = tc.tile_pool(name="qkpsum", bufs=2, space="PSUM")
            with qkpsum_cm as qkpsum:

                def proj_tasks(h, qt, kt):
                    """Micro-tasks for head h's Q^T/K^T projections: one matmul
                    (or finishing DVE op) per yield. Window-interleaved to match
                    the startup DMA arrival order (wq/xT0 first, then wk, then
                    later xT windows)."""
                    for qf in range(NQF):
                        for dst, w, bias in ((qt, wq, bq_sb), (kt, wk, None)):
                            pq = qkpsum.tile([D, QF], f32, tag="pqk",
                                             name=f"pqk{h}_{qf}_{0 if bias is not None else 1}")
                            for ein in range(ET):
                                nc.tensor.matmul(
                                    pq[:],
                                    w[ein][:, h * D:(h + 1) * D],
                                    xTw[ein][qf][:],
                                    start=(ein == 0), stop=(ein == ET - 1),
                                )
                                yield
                            sl = dst[:, qf * QF:(qf + 1) * QF]
                            if bias is not None:
                                nc.vector.tensor_scalar_add(sl, pq[:],
                                                            bias[:, h:h + 1])
                            else:
                                nc.vector.tensor_copy(sl, pq[:])
                            yield

                def attention(h, qt, kt, next_tasks, epsum, opsum,
                              defer_fill_first_pair=False):
                    """Head h attention; drains next_tasks (next head's
                    projections, or the tail of the output projection) between
                    inner iterations to fill PE slack."""
                    def drain(k, qp=1):
                        if defer_fill_first_pair and qp == 0:
                            return
                        for _ in range(k):
                            if next_tasks is None:
                                return
                            if next(next_tasks, "done") == "done":
                                return

                    for qp in range(NQP):
                        po = [opsum.tile([DP, QF], f32, tag="po",
                                         name=f"po{h}_{qp}_{j}")
                              for j in range(2)]
                        for kc in range(NT):
                            pe = epsum.tile([128, 2 * QF], f32, tag="pe",
                                            name=f"pe{h}_{qp}_{kc}")
                            for j in range(2):
                                nc.tensor.matmul(
                                    pe[:, j * QF:(j + 1) * QF],
                                    kt[:, kc * 128:(kc + 1) * 128],
                                    qt[:, (2 * qp + j) * QF:(2 * qp + j + 1) * QF],
                                    start=True, stop=True,
                                )
                            att = att_pool.tile([128, 2 * QF], bf, tag="att",
                                                name=f"att{h}_{qp}_{kc}")
                            nc.scalar.activation(att[:], pe[:], Exp)
                            for j in range(2):
                                nc.tensor.matmul(
                                    po[j][:],
                                    vtiles[kc][:, h * DP:(h + 1) * DP],
                                    att[:, j * QF:(j + 1) * QF],
                                    start=(kc == 0), stop=(kc == NT - 1),
                                )
                            drain(2, qp)
                        for j in range(2):
                            qf = 2 * qp + j
                            # denominator row (po row 96) to partition 0 via a
                            # plain copy (custom-DVE ops and partition_broadcast
                            # do not honor shifted input bases on HW), then
                            # reciprocal + broadcast from partition 0.
                            den = small.tile([1, QF], f32, tag="den",
                                             name=f"den{h}_{qf}")
                            nc.vector.tensor_copy(den[0:1, :], po[j][D:DP, :])
                            rb = small.tile([1, QF], f32, tag="rb",
                                            name=f"rb{h}_{qf}")
                            nc.vector.reciprocal_approx_fast(rb[0:1, :],
                                                             den[0:1, :])
                            rbb = small.tile([128, QF], f32, tag="rbb",
                                             name=f"rbb{h}_{qf}")
                            nc.gpsimd.partition_broadcast(rbb[:], rb[0:1, :])
                            cols = slice(qf * QF, (qf + 1) * QF)
                            if h % 4 in (0, 2):
                                # pieces are quadrant-legal: normalize straight
                                # into the merged onorm tiles (1-2 DVE muls)
                                for (t, r0, r1, off) in _HEAD_PIECES[h]:
                                    cnt = r1 - r0
                                    nc.vector.tensor_mul(
                                        onorm6[t][r0:r1, cols],
                                        po[j][off:off + cnt, :],
                                        rbb[r0:r1, :])
                            else:
                                # 3-way 32-row splits would triple DVE time:
                                # one aligned DVE mul into a temp, then shifted
                                # SBUF->SBUF copies on the idle GpSimd engine
                                tmp = small.tile([D, QF], bf, tag="ntmp",
                                                 name=f"ntmp{h}_{qf}")
                                nc.vector.tensor_mul(tmp[:], po[j][0:D, :],
                                                     rbb[0:D, :])
                                for (t, r0, r1, off) in _HEAD_PIECES[h]:
                                    cnt = r1 - r0
                                    nc.gpsimd.tensor_copy(
                                        onorm6[t][r0:r1, cols],
                                        tmp[off:off + cnt, :])
                            drain(1, qp)

                # head 0 projections + V' phase, interleaved window-by-window
                # so the in-order PE consumes tiles in DMA arrival order
                # (wq/xT0, wk, wv, xT1, xT2, xT3).
                qts, kts = {}, {}
                qts[0] = qkpool.tile([D, N], bf, tag="qt", name="qt0")
                kts[0] = qkpool.tile([D, N], bf, tag="kt", name="kt0")
                p0 = proj_tasks(0, qts[0], kts[0])

                def drain_p0(k):
                    for _ in range(k):
                        if next(p0, "done") == "done":
                            return

                with tc.tile_pool(name="vpsum", bufs=2, space="PSUM") as vpsum:
                    def emit_v_chunk(nch):
                        pv = vpsum.tile([128, E], f32, tag="pv", name=f"pv{nch}")
                        for f0, f1 in ((0, 512), (512, 768)):
                            for ein in range(ET):
                                nc.tensor.matmul(
                                    pv[:, f0:f1],
                                    xTw[ein][nch // 4][:, (nch % 4) * 128:
                                                       (nch % 4 + 1) * 128],
                                    wv[ein][:, f0:f1],
                                    start=(ein == 0), stop=(ein == ET - 1),
                                )
                        vt = vpool.tile([128, H * DP], bf, tag=f"v{nch}",
                                        name=f"v{nch}")
                        vview = vt[:].rearrange("p (h c) -> p h c", c=DP)
                        nc.vector.memset(vview[:, :, D:DP], 1.0)
                        nc.vector.tensor_copy(
                            vview[:, :, 0:D],
                            pv[:].rearrange("p (h c) -> p h c", c=D),
                        )
                        vtiles.append(vt)

                    for w in range(NQF):
                        drain_p0(7)           # half a window group of q+k tasks
                        emit_v_chunk(4 * w)
                        emit_v_chunk(4 * w + 1)
                        drain_p0(7)
                        emit_v_chunk(4 * w + 2)
                        emit_v_chunk(4 * w + 3)
                    for _ in p0:
                        pass

                store_q = [nc.sync, nc.scalar, nc.gpsimd]

                def final_tasks(nchs):
                    """Output-projection micro-tasks: one matmul (or the
                    finishing copy/store) per yield. Full K=128 contraction
                    over the merged onorm tiles (6 matmuls per 512-wide half).
                    PSUM comes from the qkpsum pool's slots (idle once
                    projections are done)."""
                    for nch in nchs:
                        osb = outsb_pool.tile([128, E], bf, tag="osb",
                                              name=f"osb{nch}")
                        for f0, f1 in ((0, 512), (512, 768)):
                            pf = qkpsum.tile([128, f1 - f0], f32, tag="pqk",
                                             name=f"pf{nch}_{f0}")
                            for t in range(ET):
                                nc.tensor.matmul(
                                    pf[:],
                                    onorm6[t][:, nch * 128:(nch + 1) * 128],
                                    wo[t][:, f0:f1],
                                    start=(t == 0), stop=(t == ET - 1),
                                )
                                yield
                            nc.vector.tensor_copy(osb[:, f0:f1], pf[:])
                            yield
                        store_q[nch % 3].dma_start(
                            out_d.ap()[nch * 128:(nch + 1) * 128, :], osb[:])

                final_rest = None
                with tc.tile_pool(name="epsum", bufs=2, space="PSUM") as epsum, \
                     tc.tile_pool(name="opsum", bufs=2, space="PSUM") as opsum:
                    for h in range(H):
                        if h + 1 < H:
                            qts[h + 1] = qkpool.tile([D, N], bf, tag="qt",
                                                     name=f"qt{h+1}")
                            kts[h + 1] = qkpool.tile([D, N], bf, tag="kt",
                                                     name=f"kt{h+1}")
                            tasks = proj_tasks(h + 1, qts[h + 1], kts[h + 1])
                        else:
                            # last head: fill PE slack with the first half of
                            # the output projection (n-chunks 0..7 only need
                            # head-7 windows 0/1, normalized in window pair 0).
                            tasks = final_tasks(range(8))
                        attention(h, qts[h], kts[h], tasks, epsum, opsum,
                                  defer_fill_first_pair=(h + 1 == H))
                        if tasks is not None:
                            for _ in tasks:  # finish any leftovers
                                pass
                        qts.pop(h), kts.pop(h)
                    final_rest = final_tasks(range(8, NT))
                    for _ in final_rest:
                        pass


    nc.compile()
    return nc


def _get_nc():
    if "nc" not in _compiled:
        _install_ntff_hook_stub()
        _compiled["nc"] = _build()
    return _compiled["nc"]


def prepare_in_maps(x, Wq, Wk, Wv, Wo, bq):
    """Host-side prep: transpose/cast per-core inputs."""
    scale = np.float32(1.0 / math.sqrt(E))
    wq_b = np.ascontiguousarray(Wq.astype(_BF16))
    wk_b = np.ascontiguousarray(Wk.astype(_BF16))
    wv_b = np.ascontiguousarray(Wv.astype(_BF16))
    wo_b = np.ascontiguousarray((Wo.astype(np.float32) * scale).astype(_BF16))
    bq_c = np.ascontiguousarray(
        bq.astype(np.float32).reshape(H, D).T)  # [D, H], col h = head h bias
    in_maps = []
    for c in range(N_CORES):
        xt = x[c].T.astype(_BF16)  # [E, N]
        # window-tiled layout: row block (i*NQF + w) holds
        # x^T[128i:128(i+1), 512w:512(w+1)] contiguously (4KB DMA packets)
        xt_t = (xt.reshape(ET, 128, NQF, QF).transpose(0, 2, 1, 3)
                .reshape(ET * NQF * 128, QF))
        in_maps.append({
            "xT": np.ascontiguousarray(xt_t),
            "wq": wq_b, "wk": wk_b, "wv": wv_b, "wo": wo_b,
            "bq": bq_c,
        })
    return in_maps


def run(x, Wq, bq, Wk, bk, Wv, bv, Wo, bo, trace=False, **spmd_kwargs):
    """Run on hardware; returns (out [B,N,E] fp32, BassKernelResults)."""
    from concourse.bass_utils import run_bass_kernel_spmd

    nc = _get_nc()
    in_maps = prepare_in_maps(x, Wq, Wk, Wv, Wo, bq)
    res = run_bass_kernel_spmd(nc, in_maps, core_ids=list(range(N_CORES)),
                               trace=trace, **spmd_kwargs)
    scale = np.float32(1.0 / math.sqrt(E))
    bo_eff = (bo.astype(np.float32)
              + (bv.astype(np.float32) @ Wo.astype(np.float32)) * scale)
    out = np.stack([res.results[c]["out"].astype(np.float32)
                    for c in range(N_CORES)], axis=0)
    out = out + bo_eff[None, None, :]
    return out.astype(np.float32), res


def kernel(x, Wq, bq, Wk, bk, Wv, bv, Wo, bo):
    x = np.asarray(x); Wq = np.asarray(Wq); bq = np.asarray(bq)
    Wk = np.asarray(Wk); bk = np.asarray(bk); Wv = np.asarray(Wv)
    bv = np.asarray(bv); Wo = np.asarray(Wo); bo = np.asarray(bo)
    out, _ = run(x, Wq, bq, Wk, bk, Wv, bv, Wo, bo, trace=False)
    return out


# revision 17
# speedup vs baseline: 1.0388x; 1.0020x over previous
"""Multi-head attention Trainium2 kernel (Bass/Tile), 8-core data-parallel.

Problem: B=8, N=2048, E=768, H=8 heads, D=96.
  q = x@Wq+bq; k = x@Wk+bk; v = x@Wv+bv  (per batch)
  energy = q @ k^T per head; att = softmax(energy)/sqrt(E); out = (att@v)@Wo + bo

Sharding: data-parallel over batch — each of the 8 cores handles one batch
element with a full copy of the weights. No collectives.

Per-core algorithm (all matmuls bf16 with fp32 PSUM accumulation):
  - PE warmup: ~10 dummy matmuls on a zeroed tile at kernel start so the HAM
    clock-gate reaches K=8/8 (~3.4us) while the startup DMAs land; without it
    the trickle-fed projection matmuls run at 1.2 GHz for ~28us.
  - x^T [E, N] is DMA'd in (host pre-transposes + casts bf16).
  - Q^T_h = Wq_h^T @ x^T + bq_h  per head  [96, 2048]   (bias = per-partition DVE add)
  - K^T_h = Wk_h^T @ x^T                   [96, 2048]   (bk dropped: softmax shift-invariant)
  - V' [N, 8*97]: per head block = [ones column | 96 data cols (x@Wv)].
  - Per head, per 1024-wide q window pair:
      energy^T[k_chunk, q] = (K^T_h chunk)^T @ Q^T_h    -> PSUM [128, 2x512]
      att = exp(energy^T)  (one ACT instr per [128,1024]; no max subtraction
        needed: |energy| < ~20 so fp32/bf16 exp cannot overflow)
      out'^T [97, 512] += V'_h[k_chunk]^T @ att          (row 0 = softmax denominator)
      rb = 1/out'^T  (reciprocal_approx_fast; only row 0 -- the denominator --
        is consumed)
      rbb[0:97] = partition_broadcast(rb[0])  (GpSimd, HW broadcasts partition 0)
      normalized rows are written STRAIGHT INTO the merged onorm layout:
        onorm6[t] [128, 2048] bf16, t=0..5, rows = E-dim (heads packed at 96h),
        via 1-2 partition-shifted DVE muls per window (head rows straddle the
        128-row tile boundary for h not divisible by 4).
  - Next head's Q/K projection matmuls are interleaved into the attention loop
    so the in-order PE fills its slack while ACT (exp) is the local bottleneck;
    the last head's slack is filled with the first half of the output projection.
  - Final: out[n_chunk, :] = sum_t onorm6[t][:, n_chunk]^T @ (Wo/sqrt(E))[t],
    full K=128 contraction (6 matmuls per 512-wide half instead of 8 per-head
    K=97 matmuls). Output copied to bf16 SBUF, stored bf16 (host casts to f32).
  - Host adds bo_eff = bo + bv @ Wo / sqrt(E)  (exact because softmax rows sum to 1).
"""

import math
import sys
import types

import numpy as np
import ml_dtypes

B, N, E, H = 8, 2048, 768, 8
D = E // H          # 96
DP = D + 1          # 97: per-head V width incl. leading ones column
N_CORES = 8
NT = N // 128       # 16 row chunks of x / V
ET = E // 128       # 6 embedding chunks
QF = 512            # moving free-dim tile
NQF = N // QF       # 4 q windows
NQP = NQF // 2      # 2 q window pairs

_BF16 = ml_dtypes.bfloat16

_compiled = {}

# head h's 96 output rows land at E-rows [96h, 96h+96), i.e. at row
# (96h + d) % 128 of merged tile (96h + d) // 128. Partition accesses must
# obey the HW quadrant mux (spans <=32 rows start at any multiple of 32,
# <=64 only at 0/64, wider only at 0), so each head CLASS (h % 4) gets its
# own V'-block layout: V data column placed AT the po row that equals the
# final merged-onorm row, with the ones column (softmax denominator) in the
# class's spare 32-row quadrant. Every normalization mul is then perfectly
# partition-aligned (po row == onorm row). V' blocks are 128 wide — matmul
# cost is the streamed free dim, so M=128 costs the same as M=97.
VW = 128  # V'-block width per head

# per class: (vt-dst0, vt-dst1, pv-src0, pv-src1) data placement pieces
_V_PIECES = {
    0: [(0, 96, 0, 96)],
    1: [(96, 128, 0, 32), (0, 64, 32, 96)],
    2: [(64, 128, 0, 64), (0, 32, 64, 96)],
    3: [(32, 128, 0, 96)],
}
# per class: ones-column position == denominator po row
_DEN_ROW = {0: 96, 1: 64, 2: 32, 3: 0}
# per head: (onorm_tile, row_start, row_end) aligned normalization pieces
# (po rows == onorm rows by construction)
_HEAD_PIECES = []
for _h in range(H):
    _t0 = (D * _h) // 128
    _HEAD_PIECES.append({
        0: [(_t0, 0, 96)],
        1: [(_t0, 96, 128), (_t0 + 1, 0, 64)],
        2: [(_t0, 64, 128), (_t0 + 1, 0, 32)],
        3: [(_t0, 32, 64), (_t0, 64, 128)],
    }[_h % 4])


def _install_ntff_hook_stub():
    """bass_utils imports antenv.axon_hooks when tracing; provide the glue if
    the image's antenv stub lacks it (harmless when trace=False)."""
    if "antenv.axon_hooks" in sys.modules:
        return
    hook = None
    try:
        from trn_agent_boot.trn_boot import _ntff_profile_via_ctypes

        hook = _ntff_profile_via_ctypes("/opt/axon/libaxon_pjrt.so")
    except Exception:
        pass
    mod = types.ModuleType("antenv.axon_hooks")
    mod.get_axon_ntff_profile_hook = lambda: hook
    mod.set_axon_ntff_profile_hook = lambda h: None
    sys.modules["antenv.axon_hooks"] = mod


def _build():
    import concourse.tile as tile
    import concourse.bacc as bacc
    from concourse import mybir

    bf = mybir.dt.bfloat16
    f32 = mybir.dt.float32
    Exp = mybir.ActivationFunctionType.Exp

    nc = bacc.Bacc("TRN2", target_bir_lowering=False, debug=False,
                   num_devices=N_CORES)

    # xT stored window-tiled: tile (i, w) = x^T[128i:128(i+1), 512w:512(w+1)]
    # is contiguous in DRAM so DMA packets are 4KB (vs 1KB strided rows).
    xT_d = nc.dram_tensor("xT", [ET * NQF * 128, QF], bf, kind="ExternalInput")
    wq_d = nc.dram_tensor("wq", [E, E], bf, kind="ExternalInput")
    wk_d = nc.dram_tensor("wk", [E, E], bf, kind="ExternalInput")
    wv_d = nc.dram_tensor("wv", [E, E], bf, kind="ExternalInput")
    wo_d = nc.dram_tensor("wo", [E, E], bf, kind="ExternalInput")  # pre-scaled 1/sqrt(E)
    bq_d = nc.dram_tensor("bq", [D, H], f32, kind="ExternalInput")
    out_d = nc.dram_tensor("out", [N, E], bf, kind="ExternalOutput")

    with tile.TileContext(nc) as tc:
        from contextlib import ExitStack

        with ExitStack() as ctx:
            const = ctx.enter_context(tc.tile_pool(name="const", bufs=1))
            vpool = ctx.enter_context(tc.tile_pool(name="vstore", bufs=1))
            qkpool = ctx.enter_context(tc.tile_pool(name="qk", bufs=2))
            onpool = ctx.enter_context(tc.tile_pool(name="onorm", bufs=1))
            att_pool = ctx.enter_context(tc.tile_pool(name="att", bufs=3))
            small = ctx.enter_context(tc.tile_pool(name="small", bufs=4))
            outsb_pool = ctx.enter_context(tc.tile_pool(name="outsb", bufs=3))

            # ---- PE warmup: spin dummy matmuls so the HAM un-throttles the
            # PE clock (1.2 -> 2.4 GHz needs ~3.4us of sustained activity)
            # while the startup DMAs land.
            warm = const.tile([128, QF], bf, tag="warm", name="warm")
            nc.vector.memset(warm[:], 0.0)
            with tc.tile_pool(name="warmpsum", bufs=1, space="PSUM") as wpsum:
                pw = wpsum.tile([128, QF], f32, tag="pw", name="pw")
                for r in range(10):
                    nc.tensor.matmul(pw[:], warm[:, 0:128], warm[:],
                                     start=True, stop=True)

            # ---- persistent SBUF loads ----
            # Critical-path loads (wq / xT window 0) go on the two HWDGE
            # queues (sync, scalar) only; later tiles fan out to gpsimd's
            # software DGE as well. Each dma_start costs ~590ns of issue time
            # on its queue engine, so small loads are batched (bq is one DMA).
            xTw = [[const.tile([128, QF], bf, tag=f"xT{i}_{w}", name=f"xT{i}_{w}")
                    for w in range(NQF)] for i in range(ET)]

            def ld_xt(i, w, q):
                r = (i * NQF + w) * 128
                q.dma_start(xTw[i][w][:], xT_d.ap()[r:r + 128, :])

            wq = [const.tile([128, E], bf, tag=f"wq{i}", name=f"wq{i}")
                  for i in range(ET)]
            for i in range(ET):
                nc.sync.dma_start(wq[i][:], wq_d.ap()[i * 128:(i + 1) * 128, :])
                ld_xt(i, 0, nc.scalar)
            bq_sb = const.tile([D, H], f32, tag="bq", name="bq")
            nc.gpsimd.dma_start(bq_sb[:], bq_d.ap()[:, :])

            def load_w(dram, name, qs):
                tiles = []
                for i in range(ET):
                    t = const.tile([128, E], bf, tag=f"{name}{i}", name=f"{name}{i}")
                    qs[i % len(qs)].dma_start(
                        t[:], dram.ap()[i * 128:(i + 1) * 128, :])
                    tiles.append(t)
                return tiles

            wk = load_w(wk_d, "wk", [nc.sync, nc.gpsimd])
            for i in range(ET):
                ld_xt(i, 1, nc.scalar)
            wv = load_w(wv_d, "wv", [nc.sync, nc.gpsimd])
            for w in range(2, NQF):
                for i in range(ET):
                    ld_xt(i, w, [nc.scalar, nc.sync, nc.gpsimd][(i + w) % 3])
            wo = load_w(wo_d, "wo", [nc.gpsimd, nc.sync, nc.scalar])

            # ---- Phases 1+2 ----
            # onorm6: merged normalized-attention layout, rows = E dim
            onorm6 = [onpool.tile([128, N], bf, tag=f"on{t}", name=f"on{t}")
                      for t in range(ET)]
            vtiles = []
            qkpsum_cm = tc.tile_pool(name="qkpsum", bufs=2, space="PSUM")
            with qkpsum_cm as qkpsum:

                def proj_tasks(h, qt, kt):
                    """Micro-tasks for head h's Q^T/K^T projections: one matmul
                    (or finishing DVE op) per yield. Window-interleaved to match
                    the startup DMA arrival order (wq/xT0 first, then wk, then
                    later xT windows)."""
                    for qf in range(NQF):
                        for dst, w, bias in ((qt, wq, bq_sb), (kt, wk, None)):
                            pq = qkpsum.tile([D, QF], f32, tag="pqk",
                                             name=f"pqk{h}_{qf}_{0 if bias is not None else 1}")
                            for ein in range(ET):
                                nc.tensor.matmul(
                                    pq[:],
                                    w[ein][:, h * D:(h + 1) * D],
                                    xTw[ein][qf][:],
                                    start=(ein == 0), stop=(ein == ET - 1),
                                )
                                yield
                            sl = dst[:, qf * QF:(qf + 1) * QF]
                            if bias is not None:
                                nc.vector.tensor_scalar_add(sl, pq[:],
                                                            bias[:, h:h + 1])
                            else:
                                nc.vector.tensor_copy(sl, pq[:])
                            yield

                def attention(h, qt, kt, next_tasks, epsum, opsum,
                              defer_fill_first_pair=False):
                    """Head h attention; drains next_tasks (next head's
                    projections, or the tail of the output projection) between
                    inner iterations to fill PE slack."""
                    def drain(k, qp=1):
                        if defer_fill_first_pair and qp == 0:
                            return
                        for _ in range(k):
                            if next_tasks is None:
                                return
                            if next(next_tasks, "done") == "done":
                                return

                    for qp in range(NQP):
                        po = [opsum.tile([128, QF], f32, tag="po",
                                         name=f"po{h}_{qp}_{j}")
                              for j in range(2)]
                        for kc in range(NT):
                            pe = epsum.tile([128, 2 * QF], f32, tag="pe",
                                            name=f"pe{h}_{qp}_{kc}")
                            for j in range(2):
                                nc.tensor.matmul(
                                    pe[:, j * QF:(j + 1) * QF],
                                    kt[:, kc * 128:(kc + 1) * 128],
                                    qt[:, (2 * qp + j) * QF:(2 * qp + j + 1) * QF],
                                    start=True, stop=True,
                                )
                            att = att_pool.tile([128, 2 * QF], bf, tag="att",
                                                name=f"att{h}_{qp}_{kc}")
                            nc.scalar.activation(att[:], pe[:], Exp)
                            for j in range(2):
                                nc.tensor.matmul(
                                    po[j][:],
                                    vtiles[kc][:, h * VW:(h + 1) * VW],
                                    att[:, j * QF:(j + 1) * QF],
                                    start=(kc == 0), stop=(kc == NT - 1),
                                )
                            drain(2, qp)
                        dr = _DEN_ROW[h % 4]
                        for j in range(2):
                            qf = 2 * qp + j
                            # denominator row to partition 0 via a plain copy
                            # (custom-DVE ops and partition_broadcast do not
                            # honor shifted input bases on HW), then
                            # reciprocal + broadcast from partition 0.
                            rb = small.tile([1, QF], f32, tag="rb",
                                            name=f"rb{h}_{qf}")
                            if dr == 0:
                                nc.vector.reciprocal_approx_fast(
                                    rb[0:1, :], po[j][0:1, :])
                            else:
                                den = small.tile([1, QF], f32, tag="den",
                                                 name=f"den{h}_{qf}")
                                nc.vector.tensor_copy(den[0:1, :],
                                                      po[j][dr:dr + 1, :])
                                nc.vector.reciprocal_approx_fast(
                                    rb[0:1, :], den[0:1, :])
                            rbb = small.tile([128, QF], f32, tag="rbb",
                                             name=f"rbb{h}_{qf}")
                            nc.gpsimd.partition_broadcast(rbb[:], rb[0:1, :])
                            cols = slice(qf * QF, (qf + 1) * QF)
                            for (t, r0, r1) in _HEAD_PIECES[h]:
                                nc.vector.tensor_mul(
                                    onorm6[t][r0:r1, cols],
                                    po[j][r0:r1, :],
                                    rbb[r0:r1, :])
                            drain(1, qp)

                # head 0 projections + V' phase, interleaved window-by-window
                # so the in-order PE consumes tiles in DMA arrival order
                # (wq/xT0, wk, wv, xT1, xT2, xT3).
                qts, kts = {}, {}
                qts[0] = qkpool.tile([D, N], bf, tag="qt", name="qt0")
                kts[0] = qkpool.tile([D, N], bf, tag="kt", name="kt0")
                p0 = proj_tasks(0, qts[0], kts[0])

                def drain_p0(k):
                    for _ in range(k):
                        if next(p0, "done") == "done":
                            return

                with tc.tile_pool(name="vpsum", bufs=2, space="PSUM") as vpsum:
                    def emit_v_chunk(nch):
                        pv = vpsum.tile([128, E], f32, tag="pv", name=f"pv{nch}")
                        for f0, f1 in ((0, 512), (512, 768)):
                            for ein in range(ET):
                                nc.tensor.matmul(
                                    pv[:, f0:f1],
                                    xTw[ein][nch // 4][:, (nch % 4) * 128:
                                                       (nch % 4 + 1) * 128],
                                    wv[ein][:, f0:f1],
                                    start=(ein == 0), stop=(ein == ET - 1),
                                )
                        vt = vpool.tile([128, H * VW], bf, tag=f"v{nch}",
                                        name=f"v{nch}")
                        # class-periodic strided views: g = head group (0/1),
                        # inner = 4 classes x block
                        vtv = vt[:].rearrange("p (g x) -> p g x", x=4 * VW)
                        pvv = pv[:].rearrange("p (g x) -> p g x", x=4 * D)
                        for cls in range(4):
                            # ones column + junk init in one 32-wide memset
                            # (extra ones columns land in unread po rows)
                            dn = _DEN_ROW[cls]
                            nc.vector.memset(
                                vtv[:, :, cls * VW + dn:cls * VW + dn + 32],
                                1.0)
                            for (d0, d1, s0, s1) in _V_PIECES[cls]:
                                dst = vtv[:, :, cls * VW + d0:cls * VW + d1]
                                src = pvv[:, :, cls * D + s0:cls * D + s1]
                                if d1 - d0 > 64:
                                    nc.vector.tensor_copy(dst, src)
                                else:
                                    nc.scalar.copy(dst, src)
                        vtiles.append(vt)

                    for w in range(NQF):
                        drain_p0(7)           # half a window group of q+k tasks
                        emit_v_chunk(4 * w)
                        emit_v_chunk(4 * w + 1)
                        drain_p0(7)
                        emit_v_chunk(4 * w + 2)
                        emit_v_chunk(4 * w + 3)
                    for _ in p0:
                        pass

                store_q = [nc.sync, nc.scalar, nc.gpsimd]

                def final_tasks(nchs):
                    """Output-projection micro-tasks: one matmul (or the
                    finishing copy/store) per yield. Full K=128 contraction
                    over the merged onorm tiles (6 matmuls per 512-wide half).
                    PSUM comes from the qkpsum pool's slots (idle once
                    projections are done)."""
                    for nch in nchs:
                        osb = outsb_pool.tile([128, E], bf, tag="osb",
                                              name=f"osb{nch}")
                        for f0, f1 in ((0, 512), (512, 768)):
                            pf = qkpsum.tile([128, f1 - f0], f32, tag="pqk",
                                             name=f"pf{nch}_{f0}")
                            for t in range(ET):
                                nc.tensor.matmul(
                                    pf[:],
                                    onorm6[t][:, nch * 128:(nch + 1) * 128],
                                    wo[t][:, f0:f1],
                                    start=(t == 0), stop=(t == ET - 1),
                                )
                                yield
                            nc.vector.tensor_copy(osb[:, f0:f1], pf[:])
                            yield
                        store_q[nch % 3].dma_start(
                            out_d.ap()[nch * 128:(nch + 1) * 128, :], osb[:])

                final_rest = None
                with tc.tile_pool(name="epsum", bufs=2, space="PSUM") as epsum, \
                     tc.tile_pool(name="opsum", bufs=2, space="PSUM") as opsum:
                    for h in range(H):
                        if h + 1 < H:
                            qts[h + 1] = qkpool.tile([D, N], bf, tag="qt",
                                                     name=f"qt{h+1}")
                            kts[h + 1] = qkpool.tile([D, N], bf, tag="kt",
                                                     name=f"kt{h+1}")
                            tasks = proj_tasks(h + 1, qts[h + 1], kts[h + 1])
                        else:
                            # last head: fill PE slack with the first half of
                            # the output projection (n-chunks 0..7 only need
                            # head-7 windows 0/1, normalized in window pair 0).
                            tasks = final_tasks(range(8))
                        attention(h, qts[h], kts[h], tasks, epsum, opsum,
                                  defer_fill_first_pair=(h + 1 == H))
                        if tasks is not None:
                            for _ in tasks:  # finish any leftovers
                                pass
                        qts.pop(h), kts.pop(h)
                    final_rest = final_tasks(range(8, NT))
                    for _ in final_rest:
                        pass


    nc.compile()
    return nc


def _get_nc():
    if "nc" not in _compiled:
        _install_ntff_hook_stub()
        _compiled["nc"] = _build()
    return _compiled["nc"]


def prepare_in_maps(x, Wq, Wk, Wv, Wo, bq):
    """Host-side prep: transpose/cast per-core inputs."""
    scale = np.float32(1.0 / math.sqrt(E))
    wq_b = np.ascontiguousarray(Wq.astype(_BF16))
    wk_b = np.ascontiguousarray(Wk.astype(_BF16))
    wv_b = np.ascontiguousarray(Wv.astype(_BF16))
    wo_b = np.ascontiguousarray((Wo.astype(np.float32) * scale).astype(_BF16))
    bq_c = np.ascontiguousarray(
        bq.astype(np.float32).reshape(H, D).T)  # [D, H], col h = head h bias
    in_maps = []
    for c in range(N_CORES):
        xt = x[c].T.astype(_BF16)  # [E, N]
        # window-tiled layout: row block (i*NQF + w) holds
        # x^T[128i:128(i+1), 512w:512(w+1)] contiguously (4KB DMA packets)
        xt_t = (xt.reshape(ET, 128, NQF, QF).transpose(0, 2, 1, 3)
                .reshape(ET * NQF * 128, QF))
        in_maps.append({
            "xT": np.ascontiguousarray(xt_t),
            "wq": wq_b, "wk": wk_b, "wv": wv_b, "wo": wo_b,
            "bq": bq_c,
        })
    return in_maps


def run(x, Wq, bq, Wk, bk, Wv, bv, Wo, bo, trace=False, **spmd_kwargs):
    """Run on hardware; returns (out [B,N,E] fp32, BassKernelResults)."""
    from concourse.bass_utils import run_bass_kernel_spmd

    nc = _get_nc()
    in_maps = prepare_in_maps(x, Wq, Wk, Wv, Wo, bq)
    res = run_bass_kernel_spmd(nc, in_maps, core_ids=list(range(N_CORES)),
                               trace=trace, **spmd_kwargs)
    scale = np.float32(1.0 / math.sqrt(E))
    bo_eff = (bo.astype(np.float32)
              + (bv.astype(np.float32) @ Wo.astype(np.float32)) * scale)
    out = np.stack([res.results[c]["out"].astype(np.float32)
                    for c in range(N_CORES)], axis=0)
    out = out + bo_eff[None, None, :]
    return out.astype(np.float32), res


def kernel(x, Wq, bq, Wk, bk, Wv, bv, Wo, bo):
    x = np.asarray(x); Wq = np.asarray(Wq); bq = np.asarray(bq)
    Wk = np.asarray(Wk); bk = np.asarray(bk); Wv = np.asarray(Wv)
    bv = np.asarray(bv); Wo = np.asarray(Wo); bo = np.asarray(bo)
    out, _ = run(x, Wq, bq, Wk, bk, Wv, bv, Wo, bo, trace=False)
    return out
